# revision 10
# baseline (speedup 1.0000x reference)
"""Trainium2 Bass kernel for nn_BinarizedLinear:
    out = sign(input_b @ sign(weight).T)
with input_b (8192, 2048) and weight (2048, 2048), entries all +/-1.0 fp32.

All values are +/-1, exactly representable in fp8e4, and the linear output is
a sum of 2048 +/-1 terms -> an even integer in [-2048, 2048], so
sign(v) == clamp(v, -1, 1) exactly and fp8 operands with fp32 PSUM
accumulation are bit-exact.

v2 strategy (vs v1's fp32-DMA + on-device PE transposes, 120.8us):
the host (numpy) casts both operands to fp8 and pre-permutes them into the
k-major tiled layout the TensorEngine wants ([128 partitions, k-tile, free]),
so the device kernel is nothing but:
  - 16 big contiguous DMAs streaming x (2MB) and W (4MB) fp8 k-tiles into
    SBUF on the sync HWDGE ring, interleaved so each DoubleRow pass's
    operands land just-in-time,
  - 256 fp8 matmuls with perf_mode=DoubleRow (2 k-tiles per pass),
    accumulating k=2048 into PSUM fp32 -- exact since products are +/-1,
  - sign() fused into the PSUM->SBUF eviction as one DVE tensor_scalar
    (min 1.0 then max -1.0) casting straight to fp8,
  - fp8 DMA out (2MB; host casts back to fp32).
Per-core HBM traffic drops 32MB -> 8MB and the PE runs zero transposes.
Scheduling: the first two b-tile groups are interleaved q-major across all
8 PSUM banks so the DMA-paced ramp hides two groups' matmuls instead of
one; remaining groups run sequentially, each overlapping the previous
group's eviction+store.  A short dummy-matmul burst at the top flips the
PE's HAM clock gate to full rate during the initial DMA window.
"""

import numpy as np

BATCH, IN_LEN, OUT_LEN = 8192, 2048, 2048
N_CORES = 8
SHARD = BATCH // N_CORES  # 1024
P = 128
KT = IN_LEN // P          # 16 k-tiles (contraction)
BT = SHARD // P           # 8 b-tiles per core
OB = OUT_LEN // 512       # 4 512-wide output blocks

_cache = {}


def build_kernel(shard=SHARD, in_len=IN_LEN, out_len=OUT_LEN):
    import concourse.mybir as mybir
    import concourse.tile as tile
    from concourse import bacc

    f32 = mybir.dt.float32
    bf16 = mybir.dt.bfloat16
    fp8 = mybir.dt.float8e4

    kt = in_len // P
    bt_n = shard // P
    ob_n = out_len // 512
    kp = kt // 2  # DoubleRow passes

    nc = bacc.Bacc(None, target_bir_lowering=False)
    # host-pre-tiled fp8, k-major: xt[p, kt, b] = x[b, kt*128+p],
    # wt[p, kt, o] = sign(w)[o, kt*128+p]
    xt = nc.dram_tensor("xt", [P, kt, shard], fp8, kind="ExternalInput")
    wt = nc.dram_tensor("wt", [P, kt, out_len], fp8, kind="ExternalInput")
    out = nc.dram_tensor("out", [shard, out_len], fp8, kind="ExternalOutput")
    scratch = nc.dram_tensor("scratch", [1, 1], f32, kind="ExternalOutput")

    DR = mybir.MatmulPerfMode.DoubleRow

    with tile.TileContext(nc) as tc:
        with (
            tc.tile_pool(name="const", bufs=1) as const_pool,
            tc.tile_pool(name="xt", bufs=1) as xt_pool,
            tc.tile_pool(name="wt", bufs=1) as wt_pool,
            tc.tile_pool(name="outs", bufs=3) as out_pool,
            tc.tile_pool(name="mpsum", bufs=8, space="PSUM") as mpsum_pool,
        ):
            xt_sb = xt_pool.tile([P, kt, shard], fp8, name="xt_sb")
            wt_sb = wt_pool.tile([P, kt, out_len], fp8, name="wt_sb")

            # HAM warmup: the PE would otherwise sit idle through the
            # preamble + first DMA and run at half clock (K=4/8) for its
            # first ~3.4us of real work; a dummy burst during the DMA
            # window flips the gate early.
            warm_src = const_pool.tile([P, 512], bf16, name="warm_src")
            nc.gpsimd.memset(warm_src[:], 1.0)
            warm_psum = mpsum_pool.tile([P, 512], f32, name="warm_psum",
                                        tag="mp")
            WARM = 5
            for i in range(WARM):
                nc.tensor.matmul(
                    warm_psum[:], warm_src[:, :P], warm_src[:],
                    start=(i == 0), stop=(i == WARM - 1),
                )
            warm_out = const_pool.tile([1, 1], f32, name="warm_out")
            nc.vector.tensor_copy(out=warm_out[:], in_=warm_psum[:1, :1])
            nc.gpsimd.dma_start(out=scratch[:], in_=warm_out[:])

            # input stream: one HWDGE ring, emitted in exactly the order
            # the matmul passes consume k-pairs.  The ramp (b-tiles 0-1)
            # only needs x columns 0:256, so per pass we ship w (512KB) +
            # that x slice (64KB) = 333GB/s of demand, under the 358GB/s
            # HBM limit -> the ramp stays compute-paced; the remaining x
            # columns stream afterwards, arriving just ahead of b-tile 2+.
            RAMP_B = 2 * P  # x columns needed by the ramp groups
            for q in range(kp):
                nc.sync.dma_start(
                    out=wt_sb[:, 2 * q:2 * q + 2, :],
                    in_=wt[:, 2 * q:2 * q + 2, :],
                )
                nc.sync.dma_start(
                    out=xt_sb[:, 2 * q:2 * q + 2, :RAMP_B],
                    in_=xt[:, 2 * q:2 * q + 2, :RAMP_B],
                )
            for h in range(2):
                nc.sync.dma_start(
                    out=xt_sb[:, h * (kt // 2):(h + 1) * (kt // 2), RAMP_B:],
                    in_=xt[:, h * (kt // 2):(h + 1) * (kt // 2), RAMP_B:],
                )

            def mm_pass(psums, b, q):
                for ob in range(ob_n):
                    nc.tensor.matmul(
                        psums[ob][:],
                        xt_sb[:, 2 * q:2 * q + 2, b * P:(b + 1) * P],
                        wt_sb[:, 2 * q:2 * q + 2, ob * 512:(ob + 1) * 512],
                        start=(q == 0),
                        stop=(q == kp - 1),
                        perf_mode=DR,
                    )

            def evict(psums, b, out_eng):
                ot = out_pool.tile([P, out_len], fp8, name=f"ot{b}", tag="ot")
                for ob in range(ob_n):
                    # sign(v) for even integer v: clamp to [-1, 1]
                    nc.vector.tensor_scalar(
                        out=ot[:, ob * 512:(ob + 1) * 512], in0=psums[ob][:],
                        scalar1=1.0, scalar2=-1.0,
                        op0=mybir.AluOpType.min, op1=mybir.AluOpType.max,
                    )
                out_eng.dma_start(
                    out=out[b * P:(b + 1) * P, :], in_=ot[:])

            # ramp: b-tiles 0 and 1 interleaved q-major across all 8 PSUM
            # banks -- both are paced by the input DMA stream anyway.
            ps0 = [mpsum_pool.tile([P, 512], f32, name=f"ps0_{i}", tag="mp")
                   for i in range(ob_n)]
            ps1 = [mpsum_pool.tile([P, 512], f32, name=f"ps1_{i}", tag="mp")
                   for i in range(ob_n)]
            for q in range(kp):
                mm_pass(ps0, 0, q)
                mm_pass(ps1, 1, q)
            evict(ps0, 0, nc.gpsimd)
            evict(ps1, 1, nc.gpsimd)

            # steady state: one b-tile at a time; its matmuls overlap the
            # previous tile's eviction + store.
            for b in range(2, bt_n - 1):
                ps = [mpsum_pool.tile([P, 512], f32, name=f"ps{b}_{i}", tag="mp")
                      for i in range(ob_n)]
                for q in range(kp):
                    mm_pass(ps, b, q)
                evict(ps, b, nc.gpsimd)

            # final b-tile: its eviction + store are the kernel's tail, so
            # split them across DVE/ACT and the two HWDGE rings.
            b = bt_n - 1
            ps = [mpsum_pool.tile([P, 512], f32, name=f"ps{b}_{i}", tag="mp")
                  for i in range(ob_n)]
            for q in range(kp):
                mm_pass(ps, b, q)
            ot = out_pool.tile([P, out_len], fp8, name="ot_last", tag="ot")
            for ob in (0, 1):
                nc.vector.tensor_scalar(
                    out=ot[:, ob * 512:(ob + 1) * 512], in0=ps[ob][:],
                    scalar1=1.0, scalar2=-1.0,
                    op0=mybir.AluOpType.min, op1=mybir.AluOpType.max,
                )
            for ob in (2, 3):
                # ACT's Sign table: sign(v), 0 -> 0, same as the DVE clamp
                nc.scalar.sign(
                    out=ot[:, ob * 512:(ob + 1) * 512], in_=ps[ob][:])
            half = out_len // 2
            nc.sync.dma_start(
                out=out[b * P:(b + 1) * P, :half], in_=ot[:, :half])
            nc.scalar.dma_start(
                out=out[b * P:(b + 1) * P, half:], in_=ot[:, half:])

    nc.finalize()
    return nc


def _get_nc():
    if "nc" not in _cache:
        _cache["nc"] = build_kernel()
    return _cache["nc"]


def _tile_kmajor(a2d, n_rows, kt=KT):
    """[rows, k] fp8 -> contiguous [128, kt, rows] with [p, t, r] = a[r, t*128+p]."""
    return np.ascontiguousarray(a2d.reshape(n_rows, kt, P).transpose(2, 1, 0))


def run_sharded(input_b, weight, trace=False):
    """Run the SPMD kernel; returns (output fp32, BassKernelResults)."""
    import ml_dtypes
    from concourse.bass_utils import run_bass_kernel_spmd

    fp8 = ml_dtypes.float8_e4m3
    nc = _get_nc()

    x8 = np.asarray(input_b, dtype=np.float32).astype(fp8)
    w8 = np.sign(np.asarray(weight, dtype=np.float32)).astype(fp8)
    wt = _tile_kmajor(w8, OUT_LEN)
    in_maps = [
        {"xt": _tile_kmajor(x8[c * SHARD:(c + 1) * SHARD], SHARD), "wt": wt}
        for c in range(N_CORES)
    ]
    res = run_bass_kernel_spmd(nc, in_maps, list(range(N_CORES)), trace=trace)
    out = np.concatenate([res.results[c]["out"] for c in range(N_CORES)], axis=0)
    return out.astype(np.float32), res


def kernel(input_b, weight):
    out, _ = run_sharded(input_b, weight, trace=False)
    return out


# revision 11
# speedup vs baseline: 1.1820x; 1.1820x over previous
"""Trainium2 Bass kernel for nn_BinarizedLinear:
    out = sign(input_b @ sign(weight).T)
with input_b (8192, 2048) and weight (2048, 2048), entries all +/-1.0 fp32.

All values are +/-1, exactly representable in fp8e4, and the linear output is
a sum of 2048 +/-1 terms -> an even integer in [-2048, 2048], so
sign(v) == clamp(v, -1, 1) exactly and fp8 operands with fp32 PSUM
accumulation are bit-exact.

v2 strategy (vs v1's fp32-DMA + on-device PE transposes, 120.8us):
the host (numpy) casts both operands to fp8 and pre-permutes them into the
k-major tiled layout the TensorEngine wants ([128 partitions, k-tile, free]),
so the device kernel is nothing but:
  - 16 big contiguous DMAs streaming x (2MB) and W (4MB) fp8 k-tiles into
    SBUF on the sync HWDGE ring, interleaved so each DoubleRow pass's
    operands land just-in-time,
  - 256 fp8 matmuls with perf_mode=DoubleRow (2 k-tiles per pass),
    accumulating k=2048 into PSUM fp32 -- exact since products are +/-1,
  - sign() fused into the PSUM->SBUF eviction as one DVE tensor_scalar
    (min 1.0 then max -1.0) casting straight to fp8,
  - fp8 DMA out (2MB; host casts back to fp32).
Per-core HBM traffic drops 32MB -> 8MB and the PE runs zero transposes.
Scheduling: the first two b-tile groups are interleaved q-major across all
8 PSUM banks so the DMA-paced ramp hides two groups' matmuls instead of
one; remaining groups run sequentially, each overlapping the previous
group's eviction+store.  A short dummy-matmul burst at the top flips the
PE's HAM clock gate to full rate during the initial DMA window.
"""

import numpy as np

BATCH, IN_LEN, OUT_LEN = 8192, 2048, 2048
N_CORES = 8
SHARD = BATCH // N_CORES  # 1024
P = 128
KT = IN_LEN // P          # 16 k-tiles (contraction)
BT = SHARD // P           # 8 b-tiles per core
OB = OUT_LEN // 512       # 4 512-wide output blocks

_cache = {}


def build_kernel(shard=SHARD, in_len=IN_LEN, out_len=OUT_LEN):
    import concourse.mybir as mybir
    import concourse.tile as tile
    from concourse import bacc

    f32 = mybir.dt.float32
    bf16 = mybir.dt.bfloat16
    fp8 = mybir.dt.float8e4

    kt = in_len // P
    bt_n = shard // P
    ob_n = out_len // 512
    kp = kt // 2  # DoubleRow passes

    nc = bacc.Bacc(None, target_bir_lowering=False)
    # host-pre-tiled fp8, k-major: xt[p, kt, b] = x[b, kt*128+p],
    # wt[p, kt, o] = sign(w)[o, kt*128+p]
    xt = nc.dram_tensor("xt", [P, kt, shard], fp8, kind="ExternalInput")
    wt = nc.dram_tensor("wt", [P, kt, out_len], fp8, kind="ExternalInput")
    out = nc.dram_tensor("out", [shard, out_len], fp8, kind="ExternalOutput")
    scratch = nc.dram_tensor("scratch", [1, 1], f32, kind="ExternalOutput")

    DR = mybir.MatmulPerfMode.DoubleRow

    with tile.TileContext(nc) as tc:
        with (
            tc.tile_pool(name="const", bufs=1) as const_pool,
            tc.tile_pool(name="xt", bufs=1) as xt_pool,
            tc.tile_pool(name="wt", bufs=1) as wt_pool,
            tc.tile_pool(name="outs", bufs=3) as out_pool,
            tc.tile_pool(name="mpsum", bufs=8, space="PSUM") as mpsum_pool,
        ):
            xt_sb = xt_pool.tile([P, kt, shard], fp8, name="xt_sb")
            wt_sb = wt_pool.tile([P, kt, out_len], fp8, name="wt_sb")

            # HAM warmup: the PE would otherwise sit idle through the
            # preamble + first DMA and run at half clock (K=4/8) for its
            # first ~3.4us of real work; a dummy burst during the DMA
            # window flips the gate early.
            warm_src = const_pool.tile([P, 512], bf16, name="warm_src")
            nc.gpsimd.memset(warm_src[:], 1.0)
            warm_psum = mpsum_pool.tile([P, 512], f32, name="warm_psum",
                                        tag="mp")
            WARM = 5
            for i in range(WARM):
                nc.tensor.matmul(
                    warm_psum[:], warm_src[:, :P], warm_src[:],
                    start=(i == 0), stop=(i == WARM - 1),
                )
            warm_out = const_pool.tile([1, 1], f32, name="warm_out")
            nc.vector.tensor_copy(out=warm_out[:], in_=warm_psum[:1, :1])
            nc.gpsimd.dma_start(out=scratch[:], in_=warm_out[:])

            # input stream: one HWDGE ring, emitted in exactly the order
            # the matmul passes consume k-pairs.  The ramp (b-tiles 0-1)
            # only needs x columns 0:256, so per pass we ship w (512KB) +
            # that x slice (64KB) = 333GB/s of demand, under the 358GB/s
            # HBM limit -> the ramp stays compute-paced; the remaining x
            # columns stream afterwards, arriving just ahead of b-tile 2+.
            RAMP_B = 2 * P  # x columns needed by the ramp groups
            for q in range(kp):
                nc.sync.dma_start(
                    out=wt_sb[:, 2 * q:2 * q + 2, :],
                    in_=wt[:, 2 * q:2 * q + 2, :],
                )
                nc.sync.dma_start(
                    out=xt_sb[:, 2 * q:2 * q + 2, :RAMP_B],
                    in_=xt[:, 2 * q:2 * q + 2, :RAMP_B],
                )
            for q in range(kp):
                nc.sync.dma_start(
                    out=xt_sb[:, 2 * q:2 * q + 2, RAMP_B:],
                    in_=xt[:, 2 * q:2 * q + 2, RAMP_B:],
                )

            def mm_pass(psums, b, q):
                for ob in range(ob_n):
                    nc.tensor.matmul(
                        psums[ob][:],
                        xt_sb[:, 2 * q:2 * q + 2, b * P:(b + 1) * P],
                        wt_sb[:, 2 * q:2 * q + 2, ob * 512:(ob + 1) * 512],
                        start=(q == 0),
                        stop=(q == kp - 1),
                        perf_mode=DR,
                    )

            def evict(psums, b, out_eng):
                ot = out_pool.tile([P, out_len], fp8, name=f"ot{b}", tag="ot")
                for ob in range(ob_n):
                    # sign(v) for even integer v: clamp to [-1, 1]
                    nc.vector.tensor_scalar(
                        out=ot[:, ob * 512:(ob + 1) * 512], in0=psums[ob][:],
                        scalar1=1.0, scalar2=-1.0,
                        op0=mybir.AluOpType.min, op1=mybir.AluOpType.max,
                    )
                out_eng.dma_start(
                    out=out[b * P:(b + 1) * P, :], in_=ot[:])

            # ramp: b-tiles 0 and 1 interleaved q-major across all 8 PSUM
            # banks -- both are paced by the input DMA stream anyway.
            ps0 = [mpsum_pool.tile([P, 512], f32, name=f"ps0_{i}", tag="mp")
                   for i in range(ob_n)]
            ps1 = [mpsum_pool.tile([P, 512], f32, name=f"ps1_{i}", tag="mp")
                   for i in range(ob_n)]
            for q in range(kp):
                mm_pass(ps0, 0, q)
                mm_pass(ps1, 1, q)
            evict(ps0, 0, nc.gpsimd)
            evict(ps1, 1, nc.gpsimd)

            # steady state: one b-tile at a time; its matmuls overlap the
            # previous tile's eviction + store.
            for b in range(2, bt_n - 1):
                ps = [mpsum_pool.tile([P, 512], f32, name=f"ps{b}_{i}", tag="mp")
                      for i in range(ob_n)]
                for q in range(kp):
                    mm_pass(ps, b, q)
                evict(ps, b, nc.gpsimd)

            # final b-tile: its eviction + store are the kernel's tail, so
            # split them across DVE/ACT and the two HWDGE rings.
            b = bt_n - 1
            ps = [mpsum_pool.tile([P, 512], f32, name=f"ps{b}_{i}", tag="mp")
                  for i in range(ob_n)]
            for q in range(kp):
                mm_pass(ps, b, q)
            ot = out_pool.tile([P, out_len], fp8, name="ot_last", tag="ot")
            for ob in (0, 1):
                nc.vector.tensor_scalar(
                    out=ot[:, ob * 512:(ob + 1) * 512], in0=ps[ob][:],
                    scalar1=1.0, scalar2=-1.0,
                    op0=mybir.AluOpType.min, op1=mybir.AluOpType.max,
                )
            for ob in (2, 3):
                # ACT's Sign table: sign(v), 0 -> 0, same as the DVE clamp
                nc.scalar.sign(
                    out=ot[:, ob * 512:(ob + 1) * 512], in_=ps[ob][:])
            half = out_len // 2
            nc.sync.dma_start(
                out=out[b * P:(b + 1) * P, :half], in_=ot[:, :half])
            nc.scalar.dma_start(
                out=out[b * P:(b + 1) * P, half:], in_=ot[:, half:])

    nc.finalize()
    return nc


def _get_nc():
    if "nc" not in _cache:
        _cache["nc"] = build_kernel()
    return _cache["nc"]


def _tile_kmajor(a2d, n_rows, kt=KT):
    """[rows, k] fp8 -> contiguous [128, kt, rows] with [p, t, r] = a[r, t*128+p]."""
    return np.ascontiguousarray(a2d.reshape(n_rows, kt, P).transpose(2, 1, 0))


def run_sharded(input_b, weight, trace=False):
    """Run the SPMD kernel; returns (output fp32, BassKernelResults)."""
    import ml_dtypes
    from concourse.bass_utils import run_bass_kernel_spmd

    fp8 = ml_dtypes.float8_e4m3
    nc = _get_nc()

    x8 = np.asarray(input_b, dtype=np.float32).astype(fp8)
    w8 = np.sign(np.asarray(weight, dtype=np.float32)).astype(fp8)
    wt = _tile_kmajor(w8, OUT_LEN)
    in_maps = [
        {"xt": _tile_kmajor(x8[c * SHARD:(c + 1) * SHARD], SHARD), "wt": wt}
        for c in range(N_CORES)
    ]
    res = run_bass_kernel_spmd(nc, in_maps, list(range(N_CORES)), trace=trace)
    out = np.concatenate([res.results[c]["out"] for c in range(N_CORES)], axis=0)
    return out.astype(np.float32), res


def kernel(input_b, weight):
    out, _ = run_sharded(input_b, weight, trace=False)
    return out
